# revision 23
# baseline (speedup 1.0000x reference)
"""Causal self-attention (S=2048, B=2, D=1024, H=16) on 8 TRN2 NeuronCores.

Sharding: megatron-style head parallelism. Each core owns 2 heads (128 of the
1024 model dims): Wq/Wk/Wv column-sharded, Wo row-sharded; every core reads the
full x, computes its heads' attention plus its partial output projection, and
the host sums the 8 partial outputs.

v2: all-bf16 dataflow (halves HBM traffic, enables FWL weight loads), score
matmuls packed per head-pair on PE row-strips (concurrent via tile_position
(0,0)/(64,0)), V projected directly in [token, dim] orientation (no PE
transposes), two-head-fused exp on ACT, triangular mask only on the 128-wide
diagonal band, softmax normalization via Pool broadcast, and a hand-scheduled
PE instruction stream that interleaves projection/output-projection matmuls
into the attention pipeline so the PE never idles (keeps the HAM clock gate
at 2.4 GHz).
"""

import numpy as np

import concourse.bass as bass
import concourse.mybir as mybir
import concourse.tile as tile
from concourse import bacc, bass_utils

S, B, D = 2048, 2, 1024
NCORES = 8
HPC = 2                # heads per core
HD = 64
TOK = S * B            # 4096 tokens, batch-major: token = b*S + s
KCH = D // 128         # 8 contraction chunks for the projections
ACH = 512              # phase A token-chunk width
NAC = TOK // ACH       # 8 token chunks (4 per batch)
SCH = 512              # s-chunk width (phase B)
NSC = S // SCH         # 4 s-chunks per batch
TBLK = 128             # t-block width
NTB = S // TBLK        # 16 t-blocks per batch

F32 = mybir.dt.float32
BF16 = mybir.dt.bfloat16


def build_program():
    nc = bacc.Bacc("TRN2", target_bir_lowering=False, debug=False, num_devices=NCORES,
                   num_swdge_queues=4)

    xT = nc.dram_tensor("xT", (128, NAC, KCH, ACH), BF16, kind="ExternalInput")
    wqT = nc.dram_tensor("wqT", (128, KCH, 128), BF16, kind="ExternalInput")
    wkT = nc.dram_tensor("wkT", (128, KCH, 128), BF16, kind="ExternalInput")
    wvT = nc.dram_tensor("wvT", (128, KCH, 128), BF16, kind="ExternalInput")
    woS = nc.dram_tensor("woS", (128, KCH, 128), BF16, kind="ExternalInput")
    msk = nc.dram_tensor("msk", (128, TBLK), BF16, kind="ExternalInput")
    idn = nc.dram_tensor("idn", (128, 128), BF16, kind="ExternalInput")
    out = nc.dram_tensor("out", (TOK, D), BF16, kind="ExternalOutput")

    with tile.TileContext(nc) as tc:
        with (
            tc.tile_pool(name="persist", bufs=1) as persist,
            tc.tile_pool(name="xt", bufs=3) as xtp,
            tc.tile_pool(name="pt", bufs=6) as ptp,
            tc.tile_pool(name="small", bufs=4) as smallp,
            tc.tile_pool(name="outsb", bufs=4) as outp,
            tc.tile_pool(name="fill_ps", bufs=2, space="PSUM") as fill_ps,
            tc.tile_pool(name="st_ps", bufs=2, space="PSUM") as st_ps,
            tc.tile_pool(name="ap_ps", bufs=2, space="PSUM") as ap_ps,
        ):
            # ---- persistent tiles
            wq_sb = persist.tile([128, KCH, 128], BF16)
            wk_sb = persist.tile([128, KCH, 128], BF16)
            wv_sb = persist.tile([128, KCH, 128], BF16)
            wo_full = persist.tile([128, KCH, 128], BF16)
            wo_sb = wo_full.rearrange("p o m -> p (o m)")
            msk_sb = persist.tile([128, TBLK], BF16)
            idn_sb = persist.tile([128, 128], BF16)
            qt_c = [persist.tile([128, ACH], BF16, name=f"qt_c{g}") for g in range(NAC)]
            kt_c = [persist.tile([128, ACH], BF16, name=f"kt_c{g}") for g in range(NAC)]
            # V: [t-part, b, t-block, head, 65]; 65th col = 1.0 for row-sums
            v_sb = persist.tile([128, B, NTB, HPC, HD + 1], BF16)
            atn_sb = persist.tile([128, TOK], BF16)

            nc.sync.dma_start(wq_sb[:, 0:KCH // 2, :], wqT[:, 0:KCH // 2, :])
            nc.sync.dma_start(wq_sb[:, KCH // 2:KCH, :], wqT[:, KCH // 2:KCH, :])
            nc.gpsimd.dma_start(wk_sb, wkT[:, :, :])
            nc.gpsimd.dma_start(wv_sb, wvT[:, :, :])
            nc.gpsimd.dma_start(msk_sb, msk[:, :])
            nc.gpsimd.dma_start(idn_sb, idn[:, :])
            nc.gpsimd.dma_start(wo_full, woS[:, :, :])
            nc.vector.memset(v_sb[:, :, :, :, HD:HD + 1], 1.0)

            # ---- phase A: one chunk (512 tokens) -> list of PE-work bundles
            def phase_a_bundles(g):
                b, tc4 = divmod(g, NAC // B)
                cell = {}

                def bndl_dma():
                    xt = xtp.tile([128, KCH, ACH], BF16, tag="xt", name="xt")
                    cell["xt"] = xt
                    half = KCH // 2
                    nc.sync.dma_start(xt[:, 0:half, :], xT[:, g, 0:half, :])
                    nc.sync.dma_start(xt[:, half:KCH, :], xT[:, g, half:KCH, :])

                def bndl_q():
                    xt = cell["xt"]
                    ps = fill_ps.tile([128, ACH], F32, tag="fill", name="ps_q")
                    for o in range(KCH):
                        nc.tensor.matmul(ps, wq_sb[:, o, :], xt[:, o, :],
                                         start=(o == 0), stop=(o == KCH - 1))
                    nc.vector.tensor_copy(qt_c[g][:, :], ps)

                def bndl_k():
                    xt = cell["xt"]
                    ps = fill_ps.tile([128, ACH], F32, tag="fill", name="ps_k")
                    for o in range(KCH):
                        nc.tensor.matmul(ps, wk_sb[:, o, :], xt[:, o, :],
                                         start=(o == 0), stop=(o == KCH - 1))
                    nc.vector.tensor_copy(kt_c[g][:, :], ps)

                def bndl_v(u):
                    xt = cell["xt"]
                    vp = fill_ps.tile([128, TBLK], F32, tag="fill", name="ps_v")
                    tsl = slice(u * TBLK, (u + 1) * TBLK)
                    for o in range(KCH):
                        nc.tensor.matmul(vp, xt[:, o, tsl], wv_sb[:, o, :],
                                         start=(o == 0), stop=(o == KCH - 1))
                    jb = tc4 * (ACH // TBLK) + u
                    nc.vector.tensor_copy(
                        v_sb[:, b, jb, :, 0:HD],
                        vp.rearrange("p (h e) -> p h e", h=HPC),
                    )

                return [bndl_dma, bndl_q, bndl_k] + \
                    [lambda u=u: bndl_v(u) for u in range(ACH // TBLK)]

            # ---- phase C: one t-block output projection -> one bundle
            def phase_c_bundle(b, tk, tail=False):
                def bndl():
                    base = b * S
                    t_sl = slice(base + tk * TBLK, base + (tk + 1) * TBLK)
                    ob = outp.tile([128, D], BF16, tag="ob")
                    for n in range(D // 512):
                        c_sl = slice(n * 512, (n + 1) * 512)
                        op = fill_ps.tile([128, 512], F32, tag="fill", name="op")
                        nc.tensor.matmul(op, atn_sb[:, t_sl], wo_sb[:, c_sl],
                                         start=True, stop=True)
                        # tail blocks: ACT is idle (exps done) - run the two
                        # PSUM->SBUF copies on separate engines
                        if tail and n == 1:
                            nc.scalar.copy(ob[:, c_sl], op)
                        else:
                            nc.vector.tensor_copy(ob[:, c_sl], op)
                        # ship each half as soon as its copy lands
                        oeng = (nc.sync, nc.gpsimd)[(2 * tk + n) % 2]
                        oeng.dma_start(out[t_sl, c_sl], ob[:, c_sl])
                return bndl

            # ---- phase B: one s-chunk of attention, interleaving fillers
            def phase_b_chunk(b, i, fillers):
                jmax = (i + 1) * (SCH // TBLK)
                s_sl = slice(b * S + i * SCH, b * S + (i + 1) * SCH)
                nfill = len(fillers)
                fi = 0
                pts = {}

                def scores(j):
                    di = j - i * 4
                    off = max(di, 0) * TBLK
                    ch = b * (NAC // B) + j // 4
                    tsl = slice((j % 4) * TBLK, (j % 4 + 1) * TBLK)
                    stp = st_ps.tile([128, HPC, SCH], F32, tag="st")
                    for h in range(HPC):
                        hsl = slice(h * HD, (h + 1) * HD)
                        nc.tensor.matmul(stp[:, h, off:SCH], kt_c[ch][hsl, tsl],
                                         qt_c[b * (NAC // B) + i][hsl, off:SCH],
                                         start=True, stop=(di < 0))
                    if di >= 0:
                        # accumulate -1e5 strict-upper-tri into the diagonal
                        # band on the PE: exp then yields exact zeros there
                        for h in range(HPC):
                            nc.tensor.matmul(stp[:, h, off:off + TBLK],
                                             idn_sb[:, :], msk_sb[:, :],
                                             start=False, stop=True,
                                             skip_group_check=True)
                    pt = ptp.tile([128, HPC, SCH], BF16, tag="pt")
                    nc.scalar.activation(pt[:, :, off:SCH], stp[:, :, off:SCH],
                                         mybir.ActivationFunctionType.Exp,
                                         scale=0.125)
                    pts[j] = (pt, off)

                def pv(j, aps):
                    pt, off = pts[j]
                    for h in range(HPC):
                        nc.tensor.matmul(aps[h][:, off:SCH], v_sb[:, b, j, h, :],
                                         pt[:, h, off:SCH],
                                         start=(j == 0), stop=(j == jmax - 1))
                    del pts[j]

                aps = [ap_ps.tile([HD + 1, SCH], F32, tag="ap", name=f"ap{h}")
                       for h in range(HPC)]
                for jj in range(0, jmax, 2):
                    # burst of two score pairs: each pair's LDWEIGHTS hides
                    # under the other pair's opposite-row-strip matmul
                    scores(jj)
                    scores(jj + 1)
                    if jj >= 2:
                        pv(jj - 2, aps)
                        pv(jj - 1, aps)
                    # dispense an even share of filler bundles at this step
                    tgt = (jj + 2) * nfill // jmax
                    while fi < tgt:
                        fillers[fi]()
                        fi += 1
                pv(jmax - 2, aps)
                pv(jmax - 1, aps)
                for h in range(HPC):
                    rs = smallp.tile([1, SCH], F32, tag="rs")
                    nc.vector.tensor_copy(rs, aps[h][HD:HD + 1, :])
                    rc = smallp.tile([1, SCH], F32, tag="rc")
                    nc.vector.reciprocal_approx_fast(rc, rs)
                    rb = smallp.tile([HD, SCH], F32, tag="rb")
                    nc.gpsimd.partition_broadcast(rb, rc)
                    nc.vector.tensor_mul(atn_sb[h * HD:(h + 1) * HD, s_sl],
                                         aps[h][0:HD, :], rb)

            # ---------------- driver ----------------
            # prologue: batch-0 projections (PE-dense, DMA-paced at the start)
            for g in range(NAC // B):
                for bd in phase_a_bundles(g):
                    bd()

            cb0 = [phase_c_bundle(0, tk) for tk in range(NTB)]
            cb1 = [phase_c_bundle(1, tk, tail=(tk >= 12)) for tk in range(NTB)]
            a4 = phase_a_bundles(4)
            a5 = phase_a_bundles(5)
            a6 = phase_a_bundles(6)
            a7 = phase_a_bundles(7)

            phase_b_chunk(0, 0, a4[0:3])            # dma, q, k of chunk 4
            phase_b_chunk(0, 1, a4[3:7] + cb0[0:4])
            phase_b_chunk(0, 2, a5[0:3] + cb0[4:8])
            phase_b_chunk(0, 3, a5[3:7] + cb0[8:12])
            phase_b_chunk(1, 0, a6[0:3] + cb0[12:14])
            phase_b_chunk(1, 1, a6[3:7] + a7[0:2] + cb0[14:16])
            phase_b_chunk(1, 2, a7[2:7] + cb1[0:2])
            phase_b_chunk(1, 3, cb1[2:12])
            for bd in cb1[12:16]:
                bd()

    nc.compile()
    return nc


_CACHE = {}


def _get_program():
    if "nc" not in _CACHE:
        _CACHE["nc"] = build_program()
    return _CACHE["nc"]


def _prep_in_maps(x, Wq, Wk, Wv, Wo):
    import ml_dtypes
    bf16 = ml_dtypes.bfloat16

    x = np.asarray(x, dtype=np.float32)
    Wq = np.asarray(Wq, dtype=np.float32)
    Wk = np.asarray(Wk, dtype=np.float32)
    Wv = np.asarray(Wv, dtype=np.float32)
    Wo = np.asarray(Wo, dtype=np.float32)

    # x: (S, B, D) -> xH[p, g, o, tl] = x[s, b, o*128+p], token g*ACH+tl = b*S+s
    xH = np.ascontiguousarray(
        x.transpose(2, 1, 0).reshape(KCH, 128, NAC, ACH).transpose(1, 2, 0, 3)
    ).astype(bf16)

    # additive causal bias for the 128-wide diagonal band: -1e5 where t > s
    p_idx = np.arange(128)[:, None]
    f_idx = np.arange(TBLK)[None, :]
    mskA = np.where(p_idx > f_idx, -1.0e5, 0.0).astype(bf16)
    idnA = np.eye(128, dtype=np.float32).astype(bf16)

    in_maps = []
    for c in range(NCORES):
        sl = slice(c * 128, (c + 1) * 128)
        in_maps.append({
            "xT": xH,
            "wqT": np.ascontiguousarray(
                Wq[sl, :].T.reshape(KCH, 128, 128).transpose(1, 0, 2)).astype(bf16),
            "wkT": np.ascontiguousarray(
                Wk[sl, :].T.reshape(KCH, 128, 128).transpose(1, 0, 2)).astype(bf16),
            "wvT": np.ascontiguousarray(
                Wv[sl, :].T.reshape(KCH, 128, 128).transpose(1, 0, 2)).astype(bf16),
            "woS": np.ascontiguousarray(Wo[:, sl].T.reshape(128, KCH, 128)).astype(bf16),
            "msk": mskA,
            "idn": idnA,
        })
    return in_maps


def run(x, Wq, Wk, Wv, Wo, trace=False):
    nc = _get_program()
    in_maps = _prep_in_maps(x, Wq, Wk, Wv, Wo)
    res = bass_utils.run_bass_kernel_spmd(
        nc, in_maps, core_ids=list(range(NCORES)), trace=trace,
    )
    partial = np.zeros((TOK, D), dtype=np.float32)
    for c in range(NCORES):
        partial += res.results[c]["out"].astype(np.float32)
    full = partial.reshape(B, S, D).transpose(1, 0, 2)  # (S, B, D)
    return np.ascontiguousarray(full), res


def kernel(x, Wq, Wk, Wv, Wo):
    out, _ = run(x, Wq, Wk, Wv, Wo, trace=False)
    return out
